# revision 10
# baseline (speedup 1.0000x reference)
"""RGCN (mean-aggr) Trainium2 kernel, 8-core SPMD, dst-sharded. v4.

v3 two-phase gather structure, plus PAIR-GATHER phase B:
  - B_d row order is chosen so that same-(tile,sub,q) tokens sit in adjacent
    rows; phase B gathers 512 B per descriptor (elem_step=256 B,
    elem_size=512 B, overlapping-window AP) serving TWO tokens (A=row r,
    B=row r+1). Unpaired tokens ride a descriptor with the B-half weight
    zeroed. ~0.8 of tokens pair -> ~0.6x phase-B descriptor count.
  - Fixed global column structure per sweep (shared SPMD program): 12
    (heavy,light) tile-pair columns (span 256) + 4 single-heavy columns
    (span 128); 128 pair-slots per column. Per column two matmuls
    (A-half start=True, B-half stop=True) and two one-hot builds ScA/ScB.
  - ScA on DVE, ScB alternating DVE/Pool; mean drains on Act.
Output is out^T (bf16) per core in permuted dst order; host inverts.
"""

import heapq

import numpy as np
import ml_dtypes

BF16 = ml_dtypes.bfloat16

P = 128
N_NODES = 100000
N_EDGES = 600000
DIM = 128
NUM_RELS = 8
NCORES = 8

TILE_DST = 16
TILE_SLOTS = TILE_DST * NUM_RELS          # 128
NTILES = 784                              # per core
CW = NTILES * TILE_DST                    # 12544
NBINS = NCORES * NTILES                   # 6272
NSUB = 2                                  # dst subranges per core (phase A)
TPS = NTILES // NSUB                      # 392 tiles per sub
NQ = 4                                    # src windows
QW = 25088                                # src window width
SWEEP_TILES = 28
NSWEEPS = NTILES // SWEEP_TILES           # 28
HEAVY_T = 16                              # heavy tiles per sweep (cap 128)
LIGHT_T = SWEEP_TILES - HEAVY_T           # light tiles (cap 64)
NHEAVY = NSWEEPS * HEAVY_T                # per core
NLIGHT = NSWEEPS * LIGHT_T
SWEEP_SLOTS = SWEEP_TILES * TILE_SLOTS    # 3584
SWEEP_DST = SWEEP_TILES * TILE_DST        # 448
SPLIT_TILES = 12                          # aggA (3 PSUM banks)
SPLIT_SLOTS = SPLIT_TILES * TILE_SLOTS    # 1536
SPLIT_DST = SPLIT_TILES * TILE_DST        # 192
RESTB_SLOTS = SWEEP_SLOTS - SPLIT_SLOTS   # 2048 (4 banks)
SWEEPS_PER_SUB = NSWEEPS // NSUB          # 14

# fixed phase-B column structure per sweep: (new-tile lo, n_tiles)
# sideA: 6 pair columns (tiles 0..11); sideB: 6 pair cols (12..23) + 4 singles
SWEEP_COLS = (
    [(2 * k, 2) for k in range(6)]
    + [(12 + 2 * k, 2) for k in range(6)]
    + [(24 + k, 1) for k in range(4)]
)
NCOLS_SWEEP = len(SWEEP_COLS)             # 16
TOTC = NSWEEPS * NCOLS_SWEEP              # 448 columns per core

_compiled = None


def _wrap16(idx_i16):
    n = len(idx_i16)
    w = idx_i16.reshape(n // 16, 16).T
    return np.ascontiguousarray(np.tile(w, (8, 1)))


def _build_program(CAPA):
    import concourse.bacc as bacc
    import concourse.tile as tile
    from concourse import bass, mybir

    AC = CAPA // P
    BROWS = NQ * CAPA + P                 # per-sub B rows (+zero row block)

    nc = bacc.Bacc(None, target_bir_lowering=False, debug=False)
    f32 = mybir.dt.float32
    bf16 = mybir.dt.bfloat16
    i16 = mybir.dt.int16
    i32 = mybir.dt.int32

    xg_d = nc.dram_tensor("xg", [NQ * QW, P], bf16, kind="ExternalInput")
    xT_d = nc.dram_tensor("xT", [P, CW], bf16, kind="ExternalInput")
    wcat_d = nc.dram_tensor("wcat", [P, NUM_RELS * P], bf16, kind="ExternalInput")
    wroot_d = nc.dram_tensor("wroot", [P, P], bf16, kind="ExternalInput")
    bias_d = nc.dram_tensor("bias", [P, 1], f32, kind="ExternalInput")
    gA_d = nc.dram_tensor("gA", [NSUB * NQ, P, CAPA // 16], i16, kind="ExternalInput")
    gB_d = nc.dram_tensor("gB", [P, TOTC * P // 16], i16, kind="ExternalInput")
    scA_d = nc.dram_tensor("scA", [P, TOTC], f32, kind="ExternalInput")
    wgA_d = nc.dram_tensor("wgA", [P, TOTC], f32, kind="ExternalInput")
    scB_d = nc.dram_tensor("scB", [P, TOTC], f32, kind="ExternalInput")
    wgB_d = nc.dram_tensor("wgB", [P, TOTC], f32, kind="ExternalInput")
    outT_d = nc.dram_tensor("outT", [P, CW], bf16, kind="ExternalOutput")

    B_d = [nc.dram_tensor(f"B{s}", [BROWS, P], bf16) for s in range(NSUB)]

    with tile.TileContext(nc) as tc:
        with (
            tc.tile_pool(name="const", bufs=1) as cpool,
            tc.tile_pool(name="stagA", bufs=3) as poolA,
            tc.tile_pool(name="stagB", bufs=4) as poolB,
            tc.tile_pool(name="spool", bufs=64) as spool,
            tc.tile_pool(name="mpool", bufs=2) as mpool,
            tc.tile_pool(name="opool", bufs=2) as opool,
            tc.tile_pool(name="ipool", bufs=4) as ipool,
            tc.tile_pool(name="psA", bufs=1, space="PSUM") as psA,
            tc.tile_pool(name="psO", bufs=1, space="PSUM") as psO,
        ):
            wcat = cpool.tile([P, NUM_RELS * P], bf16)
            wroot = cpool.tile([P, P], bf16)
            biast = cpool.tile([P, 1], f32)
            iota_i = cpool.tile([P, 2 * P], i32)
            iota_f = cpool.tile([P, 2 * P], bf16)
            zrow = cpool.tile([P, P], bf16)
            scAt = cpool.tile([P, TOTC], f32)
            wgAt = cpool.tile([P, TOTC], f32)
            scBt = cpool.tile([P, TOTC], f32)
            wgBt = cpool.tile([P, TOTC], f32)
            gBt = cpool.tile([P, TOTC * P // 16], i16)
            xTt = cpool.tile([P, CW], bf16)

            nc.sync.dma_start(out=scAt[:], in_=scA_d[:])
            nc.sync.dma_start(out=wgAt[:], in_=wgA_d[:])
            nc.sync.dma_start(out=scBt[:], in_=scB_d[:])
            nc.sync.dma_start(out=wgBt[:], in_=wgB_d[:])
            nc.sync.dma_start(out=gBt[:], in_=gB_d[:])
            nc.sync.dma_start(out=wcat[:], in_=wcat_d[:])
            nc.sync.dma_start(out=wroot[:], in_=wroot_d[:])
            nc.sync.dma_start(out=biast[:], in_=bias_d[:])
            nc.sync.dma_start(out=xTt[:], in_=xT_d[:])
            nc.gpsimd.iota(iota_i[:], pattern=[[1, 2 * P]], base=0,
                           channel_multiplier=0)
            nc.vector.tensor_copy(out=iota_f[:], in_=iota_i[:])
            nc.vector.memset(zrow[:], 0.0)

            # ---- Phase A: src-window gathers -> B_s (contiguous p-major) ----
            for s in range(NSUB):
                nc.sync.dma_start(
                    out=B_d[s][NQ * CAPA:NQ * CAPA + P, :], in_=zrow[:])
                for q in range(NQ):
                    gA = ipool.tile([P, CAPA // 16], i16, tag="gA")
                    nc.sync.dma_start(out=gA[:], in_=gA_d[s * NQ + q])
                    stag = poolA.tile([P, AC, P], bf16, tag="stagA")
                    nc.gpsimd.dma_gather(
                        out_ap=stag[:],
                        in_ap=xg_d[QW * q:QW * (q + 1), :],
                        idxs_ap=gA[:],
                        num_idxs=CAPA, num_idxs_reg=CAPA, elem_size=P,
                        single_packet=False)
                    nc.sync.dma_start(
                        out=B_d[s][CAPA * q:CAPA * (q + 1), :].rearrange(
                            "(p a) d -> p a d", p=P),
                        in_=stag[:])

            # ---- Phase B: per-sweep pair gathers + segment + transform ----
            for s in range(NSWEEPS):
                sub = s // SWEEPS_PER_SUB
                swtok = NCOLS_SWEEP * P   # 2048 pair-slots
                stag = poolB.tile([P, NCOLS_SWEEP, 2 * P], bf16, tag="stagB")
                gBv = gBt[:, s * NCOLS_SWEEP * P // 16:
                          (s + 1) * NCOLS_SWEEP * P // 16]
                # overlapping pair view: idx r reads rows r, r+1 of B_d[sub]
                in_pair = bass.AP(
                    B_d[sub][:, :].tensor, 0,
                    [[P, NQ * CAPA + 1], [1, 2 * P]])
                nc.gpsimd.dma_gather(
                    out_ap=stag[:], in_ap=in_pair, idxs_ap=gBv,
                    num_idxs=swtok, num_idxs_reg=swtok, elem_size=2 * P,
                    elem_step=P,
                    single_packet=False)

                aggA = psA.tile([P, SPLIT_SLOTS], f32, tag="aggA")
                aggB = psA.tile([P, RESTB_SLOTS], f32, tag="aggB")
                ScAs = []
                ScBs = []
                for ci, (lo_t, n_t) in enumerate(SWEEP_COLS):
                    span = n_t * TILE_SLOTS
                    col = s * NCOLS_SWEEP + ci
                    ScA = spool.tile([P, 2 * P], bf16, tag="ScA")
                    ScB = spool.tile([P, 2 * P], bf16, tag="ScB")
                    ScAs.append(ScA)
                    ScBs.append(ScB)
                    nc.vector.tensor_scalar(
                        out=ScA[:, :span], in0=iota_f[:, 0:span],
                        scalar1=scAt[:, col:col + 1],
                        scalar2=wgAt[:, col:col + 1],
                        op0=mybir.AluOpType.is_equal,
                        op1=mybir.AluOpType.mult)
                    nc.vector.tensor_scalar(
                        out=ScB[:, :span], in0=iota_f[:, 0:span],
                        scalar1=scBt[:, col:col + 1],
                        scalar2=wgBt[:, col:col + 1],
                        op0=mybir.AluOpType.is_equal,
                        op1=mybir.AluOpType.mult)
                for ci, (lo_t, n_t) in enumerate(SWEEP_COLS):
                    span = n_t * TILE_SLOTS
                    lo = lo_t * TILE_SLOTS
                    if lo_t < SPLIT_TILES:
                        aggv = aggA[:, lo:lo + span]
                    else:
                        aggv = aggB[:, lo - SPLIT_SLOTS:lo - SPLIT_SLOTS + span]
                    nc.tensor.matmul(
                        out=aggv, lhsT=stag[:, ci, 0:P],
                        rhs=ScAs[ci][:, :span],
                        start=True, stop=False)
                    nc.tensor.matmul(
                        out=aggv, lhsT=stag[:, ci, P:2 * P],
                        rhs=ScBs[ci][:, :span],
                        start=False, stop=True)

                meanA = mpool.tile([P, SPLIT_SLOTS], bf16, tag="meanA")
                meanB = mpool.tile([P, RESTB_SLOTS], bf16, tag="meanB")
                nc.scalar.activation(
                    out=meanA[:], in_=aggA[:],
                    func=mybir.ActivationFunctionType.Identity)
                nc.scalar.activation(
                    out=meanB[:], in_=aggB[:],
                    func=mybir.ActivationFunctionType.Identity)

                dst0 = s * SWEEP_DST
                outp = psO.tile([P, SWEEP_DST], f32)
                meanA_r = meanA[:].rearrange(
                    "p (dst rel) -> p dst rel", rel=NUM_RELS)
                meanB_r = meanB[:].rearrange(
                    "p (dst rel) -> p dst rel", rel=NUM_RELS)
                for r in range(NUM_RELS):
                    nc.tensor.matmul(
                        out=outp[:, :SPLIT_DST],
                        lhsT=wcat[:, r * P:(r + 1) * P],
                        rhs=meanA_r[:, :, r],
                        start=(r == 0), stop=False)
                nc.tensor.matmul(out=outp[:, :SPLIT_DST], lhsT=wroot[:],
                                 rhs=xTt[:, dst0:dst0 + SPLIT_DST],
                                 start=False, stop=True)
                for r in range(NUM_RELS):
                    nc.tensor.matmul(
                        out=outp[:, SPLIT_DST:],
                        lhsT=wcat[:, r * P:(r + 1) * P],
                        rhs=meanB_r[:, :, r],
                        start=(r == 0), stop=False)
                nc.tensor.matmul(out=outp[:, SPLIT_DST:], lhsT=wroot[:],
                                 rhs=xTt[:, dst0 + SPLIT_DST:dst0 + SWEEP_DST],
                                 start=False, stop=True)
                oT = opool.tile([P, SWEEP_DST], bf16, tag="oT")
                nc.scalar.activation(
                    out=oT[:], in_=outp[:],
                    func=mybir.ActivationFunctionType.Identity,
                    bias=biast[:, 0:1])
                nc.sync.dma_start(out=outT_d[:, dst0:dst0 + SWEEP_DST], in_=oT[:])
    nc.compile()
    return nc


def _balance(cnt_dst):
    """Two-class LPT: top-degree dst into heavy bins (cap 128), rest into
    light bins (cap 64), 16 dst each. Returns bin_of, pos_of, loads; bins
    [0, NHBINS) heavy, rest light."""
    NHBINS = NCORES * NHEAVY
    NLBINS = NCORES * NLIGHT
    order = np.argsort(-cnt_dst, kind="stable")
    bin_of = np.empty(N_NODES, np.int64)
    pos_of = np.empty(N_NODES, np.int64)
    counts = np.zeros(NBINS, np.int32)
    loads = np.zeros(NBINS, np.int64)
    nheavy_dst = NHBINS * TILE_DST
    for part, cap in ((order[:nheavy_dst], P), (order[nheavy_dst:], 64)):
        b0 = 0 if cap == P else NHBINS
        nb = NHBINS if cap == P else NLBINS
        heap = [(0, 0, b0 + b) for b in range(nb)]
        for d in part:
            deg = int(cnt_dst[d])
            load, c, b = heapq.heappop(heap)
            assert loads[b] + deg <= cap, "two-class packing infeasible"
            bin_of[d] = b
            pos_of[d] = counts[b]
            counts[b] += 1
            loads[b] += deg
            if counts[b] < TILE_DST:
                heapq.heappush(heap, (loads[b], counts[b], b))
    return bin_of, pos_of, loads


def _prepare(x, W, W_root, bias, edge_index, edge_type):
    src = np.asarray(edge_index[0], dtype=np.int64)
    dst = np.asarray(edge_index[1], dtype=np.int64)
    rel = np.asarray(edge_type, dtype=np.int64)

    cnt_slot = np.bincount(dst * NUM_RELS + rel, minlength=N_NODES * NUM_RELS)
    w_edge = (1.0 / np.maximum(cnt_slot[dst * NUM_RELS + rel], 1)).astype(np.float32)
    cnt_dst = np.bincount(dst, minlength=N_NODES).astype(np.int64)

    bin_of, pos_of, bin_load = _balance(cnt_dst)
    NHBINS = NCORES * NHEAVY
    tile_of_bin = np.empty(NBINS, np.int64)
    core_of_bin = np.empty(NBINS, np.int64)
    hrank = np.argsort(-bin_load[:NHBINS], kind="stable")
    hslot = np.arange(NHBINS) // NCORES          # 0..NHEAVY-1
    tile_of_bin[hrank] = (hslot // HEAVY_T) * SWEEP_TILES + hslot % HEAVY_T
    core_of_bin[hrank] = np.arange(NHBINS) % NCORES
    lrank = NHBINS + np.argsort(-bin_load[NHBINS:], kind="stable")
    lslot = np.arange(NCORES * NLIGHT) // NCORES
    tile_of_bin[lrank] = ((lslot // LIGHT_T) * SWEEP_TILES + HEAVY_T
                          + lslot % LIGHT_T)
    core_of_bin[lrank] = np.arange(NCORES * NLIGHT) % NCORES

    core_of_dst = core_of_bin[bin_of]
    tile_of_dst0 = tile_of_bin[bin_of]       # heavy-first per-sweep numbering
    j_of_dst = pos_of

    e_core = core_of_dst[dst]
    e_tile0 = tile_of_dst0[dst]
    e_local = j_of_dst[dst] * NUM_RELS + rel     # slot within tile [0,128)
    q = src // QW
    e_sub = e_tile0 // TPS

    # phase A bucket caps (core, sub, q) on UNIQUE src rows
    keyA = (e_core * NSUB + e_sub) * NQ + q
    upairs = np.unique(keyA * (N_NODES + 1) + src)
    bincA = np.bincount(upairs // (N_NODES + 1), minlength=NCORES * NSUB * NQ)
    CAPA = int(-(-bincA.max() // P) * P)
    CAPA = max(CAPA, P)
    AC = CAPA // P

    xg = np.zeros((NQ * QW, P), np.float32)
    xg[:N_NODES] = np.asarray(x, np.float32)
    xg = xg.astype(BF16)
    wcat = np.ascontiguousarray(
        np.asarray(W, np.float32).transpose(1, 0, 2).reshape(P, NUM_RELS * P)
    ).astype(BF16)
    wroot = np.ascontiguousarray(np.asarray(W_root, np.float32)).astype(BF16)
    biascol = np.asarray(bias, np.float32).reshape(P, 1)

    xnp = np.asarray(x, np.float32)
    in_maps = []
    dst_tables = []
    stats = {"tokens": 0, "paired": 0}
    for c in range(NCORES):
        sel = np.nonzero(e_core == c)[0]
        csrc = src[sel]
        ctile0 = e_tile0[sel]
        clocal = e_local[sel]
        cw = w_edge[sel]
        cq = q[sel]
        csub = ctile0 // TPS

        ordT = np.lexsort((cq, csrc, ctile0))
        tT = ctile0[ordT]
        qT = cq[ordT]
        sT = csub[ordT]
        rT = csrc[ordT]
        lT = clocal[ordT]
        wT = cw[ordT]
        n = len(ordT)
        # tile group boundaries
        bnd = np.nonzero(np.diff(tT, prepend=-1))[0].tolist() + [n]

        # ---- matching: per (tile, q) pair distinct unpaired rows ----------
        partner = {}                       # (sub, q, row) -> partner row
        pair_lists = [[] for _ in range(NSUB * NQ)]
        for gi in range(len(bnd) - 1):
            a, b = bnd[gi], bnd[gi + 1]
            sb = int(sT[a])
            for qq in range(NQ):
                cand = []
                seenr = set()
                for k in range(a, b):
                    if qT[k] != qq:
                        continue
                    r = int(rT[k])
                    if r in seenr:
                        continue
                    seenr.add(r)
                    if (sb, qq, r) not in partner:
                        cand.append(r)
                for k2 in range(0, len(cand) - 1, 2):
                    r1, r2 = cand[k2], cand[k2 + 1]
                    partner[(sb, qq, r1)] = r2
                    partner[(sb, qq, r2)] = r1
                    pair_lists[sb * NQ + qq].append((r1, r2))

        # ---- B_d row order per bucket: pairs adjacent, then the rest ------
        brow_of = {}                       # (sub, q, row) -> brow within sub
        gA = np.zeros((NSUB * NQ, P, CAPA // 16), np.int16)
        for sb in range(NSUB):
            for qq in range(NQ):
                bidx = sb * NQ + qq
                m = (csub == sb) & (cq == qq)
                uniq = np.unique(csrc[m])
                inpair = set()
                ordered = []
                for (r1, r2) in pair_lists[bidx]:
                    ordered.append(r1)
                    ordered.append(r2)
                    inpair.add(r1)
                    inpair.add(r2)
                for r in uniq.tolist():
                    if r not in inpair:
                        ordered.append(r)
                nrow = len(ordered)
                assert nrow == len(uniq) and nrow <= CAPA, (nrow, len(uniq), CAPA)
                for jpos, r in enumerate(ordered):
                    brow_of[(sb, qq, r)] = CAPA * qq + jpos
                arr = np.zeros(CAPA, np.int64)
                arr[:nrow] = np.asarray(ordered, np.int64) - QW * qq
                ii = np.arange(CAPA)
                jj = (ii % P) * AC + ii // P
                gA[bidx] = _wrap16(arr[jj].astype(np.int16))

        # ---- slots per tile: pair-slots + singles -------------------------
        slots_by_tile = {}   # tile0 -> list of (browA, (lA,wA), (lB,wB)|None)
        for gi in range(len(bnd) - 1):
            a, b = bnd[gi], bnd[gi + 1]
            t0 = int(tT[a])
            sb = int(sT[a])
            toks = {}
            for k in range(a, b):
                toks.setdefault((int(qT[k]), int(rT[k])), []).append(k)
            slots = []
            done = set()
            for (qq, r), ks in toks.items():
                if (qq, r) in done:
                    continue
                pr = partner.get((sb, qq, r))
                mate = toks.get((qq, pr)) if pr is not None else None
                if mate is not None and (qq, pr) not in done:
                    b1 = brow_of[(sb, qq, r)]
                    b2 = brow_of[(sb, qq, pr)]
                    if b2 == b1 + 1:
                        lowks, hiks, blo = ks, mate, b1
                    elif b1 == b2 + 1:
                        lowks, hiks, blo = mate, ks, b2
                    else:
                        lowks = None
                    if lowks is not None:
                        npair = min(len(lowks), len(hiks))
                        for t in range(npair):
                            ka, kb = lowks[t], hiks[t]
                            slots.append((blo, (int(lT[ka]), float(wT[ka])),
                                          (int(lT[kb]), float(wT[kb]))))
                        for k2 in lowks[npair:]:
                            slots.append((blo, (int(lT[k2]), float(wT[k2])),
                                          None))
                        for k2 in hiks[npair:]:
                            slots.append((blo + 1 - 1 + 1,
                                          (int(lT[k2]), float(wT[k2])), None))
                        done.add((qq, r))
                        done.add((qq, pr))
                        stats["paired"] += 2 * npair
                        continue
                for k2 in ks:
                    slots.append((brow_of[(sb, qq, r)],
                                  (int(lT[k2]), float(wT[k2])), None))
                done.add((qq, r))
            slots_by_tile[t0] = slots
        stats["tokens"] += n

        # ---- per sweep: assign tiles to fixed columns, renumber -----------
        newtile_of_old = np.empty(NTILES, np.int64)
        gB_cols = np.full((TOTC, P), NQ * CAPA, np.int64)
        scA_cols = np.zeros((TOTC, P), np.float32)
        wgA_cols = np.zeros((TOTC, P), np.float32)
        scB_cols = np.zeros((TOTC, P), np.float32)
        wgB_cols = np.zeros((TOTC, P), np.float32)
        for s in range(NSWEEPS):
            base = s * SWEEP_TILES
            hv = list(range(base, base + HEAVY_T))
            lt = list(range(base + HEAVY_T, base + SWEEP_TILES))
            nsl = {t: len(slots_by_tile.get(t, [])) for t in hv + lt}
            hv.sort(key=lambda t: -nsl[t])
            lt.sort(key=lambda t: nsl[t])
            # 12 (H,L) pairs: biggest H with smallest L; 4 smallest H single
            pairs = [(hv[i], lt[i]) for i in range(LIGHT_T)]
            singles = hv[LIGHT_T:]
            for i, (th, tl) in enumerate(pairs):
                assert nsl[th] + nsl[tl] <= P, (s, nsl[th], nsl[tl])
            # column order: sideA = pairs 0..5, sideB = pairs 6..11 + singles
            coltiles = [list(pr) for pr in pairs] + [[t] for t in singles]
            newt = base
            for ci, ts in enumerate(coltiles):
                col = s * NCOLS_SWEEP + ci
                pslot = 0
                for ti, t in enumerate(ts):
                    newtile_of_old[t] = newt
                    newt += 1
                    for (browA, (la, wa), bpart) in slots_by_tile.get(t, []):
                        gB_cols[col, pslot] = browA
                        scA_cols[col, pslot] = ti * TILE_SLOTS + la
                        wgA_cols[col, pslot] = wa
                        if bpart is not None:
                            lb, wb = bpart
                            scB_cols[col, pslot] = ti * TILE_SLOTS + lb
                            wgB_cols[col, pslot] = wb
                        pslot += 1
                assert pslot <= P
            assert newt == base + SWEEP_TILES

        gB16 = _wrap16(gB_cols.reshape(-1).astype(np.int16))
        scA_arr = np.ascontiguousarray(scA_cols.T)
        wgA_arr = np.ascontiguousarray(wgA_cols.T)
        scB_arr = np.ascontiguousarray(scB_cols.T)
        wgB_arr = np.ascontiguousarray(wgB_cols.T)

        # ---- dst table with renumbered tiles ------------------------------
        mask = core_of_dst == c
        dst_ids = np.nonzero(mask)[0]
        cols_d = (newtile_of_old[tile_of_dst0[dst_ids]] * TILE_DST
                  + j_of_dst[dst_ids])
        dst_table = np.full(CW, -1, np.int64)
        dst_table[cols_d] = dst_ids
        valid = dst_table >= 0
        xT = np.zeros((P, CW), np.float32)
        xT[:, valid] = xnp[dst_table[valid]].T
        xT = xT.astype(BF16)

        in_maps.append({
            "xg": xg,
            "xT": xT,
            "wcat": wcat,
            "wroot": wroot,
            "bias": biascol,
            "gA": gA,
            "gB": gB16,
            "scA": scA_arr,
            "wgA": wgA_arr,
            "scB": scB_arr,
            "wgB": wgB_arr,
        })
        dst_tables.append(dst_table)
    return in_maps, dst_tables, CAPA, stats


LAST_EXEC_NS = None


def kernel(x, W, W_root, bias, edge_index, edge_type):
    global _compiled, LAST_EXEC_NS
    import os
    from concourse.bass_utils import run_bass_kernel_spmd

    in_maps, dst_tables, CAPA, stats = _prepare(
        x, W, W_root, bias, edge_index, edge_type)
    key = CAPA
    if _compiled is None or _compiled[0] != key:
        nc = _build_program(CAPA)
        _compiled = (key, nc)
    nc = _compiled[1]

    trace = bool(int(os.environ.get("BASS_PROFILE", "0")))
    r = run_bass_kernel_spmd(nc, in_maps, list(range(NCORES)), trace=trace)
    if trace and getattr(r, "exec_time_ns", None) is not None:
        LAST_EXEC_NS = r.exec_time_ns
    res = r.results
    out = np.empty((N_NODES, DIM), np.float32)
    for c in range(NCORES):
        outT = np.asarray(res[c]["outT"]).astype(np.float32)
        dt = dst_tables[c]
        valid = dt >= 0
        out[dt[valid]] = outT[:, valid].T
    return out


# revision 11
# speedup vs baseline: 1.0252x; 1.0252x over previous
"""RGCN (mean-aggr) Trainium2 kernel, 8-core SPMD, dst-sharded. v4.

v3 two-phase gather structure, plus PAIR-GATHER phase B:
  - B_d row order is chosen so that same-(tile,sub,q) tokens sit in adjacent
    rows; phase B gathers 512 B per descriptor (elem_step=256 B,
    elem_size=512 B, overlapping-window AP) serving TWO tokens (A=row r,
    B=row r+1). Unpaired tokens ride a descriptor with the B-half weight
    zeroed. ~0.8 of tokens pair -> ~0.6x phase-B descriptor count.
  - Fixed global column structure per sweep (shared SPMD program): 12
    (heavy,light) tile-pair columns (span 256) + 4 single-heavy columns
    (span 128); 128 pair-slots per column. Per column two matmuls
    (A-half start=True, B-half stop=True) and two one-hot builds ScA/ScB.
  - ScA on DVE, ScB alternating DVE/Pool; mean drains on Act.
Output is out^T (bf16) per core in permuted dst order; host inverts.
"""

import heapq

import numpy as np
import ml_dtypes

BF16 = ml_dtypes.bfloat16

P = 128
N_NODES = 100000
N_EDGES = 600000
DIM = 128
NUM_RELS = 8
NCORES = 8

TILE_DST = 16
TILE_SLOTS = TILE_DST * NUM_RELS          # 128
NTILES = 784                              # per core
CW = NTILES * TILE_DST                    # 12544
NBINS = NCORES * NTILES                   # 6272
NSUB = 2                                  # dst subranges per core (phase A)
TPS = NTILES // NSUB                      # 392 tiles per sub
NQ = 4                                    # src windows
QW = 25088                                # src window width
SWEEP_TILES = 28
NSWEEPS = NTILES // SWEEP_TILES           # 28
HEAVY_T = 16                              # heavy tiles per sweep (cap 128)
LIGHT_T = SWEEP_TILES - HEAVY_T           # light tiles (cap 64)
NHEAVY = NSWEEPS * HEAVY_T                # per core
NLIGHT = NSWEEPS * LIGHT_T
SWEEP_SLOTS = SWEEP_TILES * TILE_SLOTS    # 3584
SWEEP_DST = SWEEP_TILES * TILE_DST        # 448
SPLIT_TILES = 12                          # aggA (3 PSUM banks)
SPLIT_SLOTS = SPLIT_TILES * TILE_SLOTS    # 1536
SPLIT_DST = SPLIT_TILES * TILE_DST        # 192
RESTB_SLOTS = SWEEP_SLOTS - SPLIT_SLOTS   # 2048 (4 banks)
SWEEPS_PER_SUB = NSWEEPS // NSUB          # 14

# fixed phase-B column structure per sweep: (new-tile lo, n_tiles)
# sideA: 6 pair columns (tiles 0..11); sideB: 6 pair cols (12..23) + 4 singles
SWEEP_COLS = (
    [(2 * k, 2) for k in range(6)]
    + [(12 + 2 * k, 2) for k in range(6)]
    + [(24 + k, 1) for k in range(4)]
)
NCOLS_SWEEP = len(SWEEP_COLS)             # 16
TOTC = NSWEEPS * NCOLS_SWEEP              # 448 columns per core

_compiled = None


def _wrap16(idx_i16):
    n = len(idx_i16)
    w = idx_i16.reshape(n // 16, 16).T
    return np.ascontiguousarray(np.tile(w, (8, 1)))


def _build_program(CAPA):
    import concourse.bacc as bacc
    import concourse.tile as tile
    from concourse import bass, mybir

    AC = CAPA // P
    BROWS = NQ * CAPA + P                 # per-sub B rows (+zero row block)

    nc = bacc.Bacc(None, target_bir_lowering=False, debug=False)
    f32 = mybir.dt.float32
    bf16 = mybir.dt.bfloat16
    i16 = mybir.dt.int16
    i32 = mybir.dt.int32

    xg_d = nc.dram_tensor("xg", [NQ * QW, P], bf16, kind="ExternalInput")
    xT_d = nc.dram_tensor("xT", [P, CW], bf16, kind="ExternalInput")
    wcat_d = nc.dram_tensor("wcat", [P, NUM_RELS * P], bf16, kind="ExternalInput")
    wroot_d = nc.dram_tensor("wroot", [P, P], bf16, kind="ExternalInput")
    bias_d = nc.dram_tensor("bias", [P, 1], f32, kind="ExternalInput")
    gA_d = nc.dram_tensor("gA", [NSUB * NQ, P, CAPA // 16], i16, kind="ExternalInput")
    gB_d = nc.dram_tensor("gB", [P, TOTC * P // 16], i16, kind="ExternalInput")
    scA_d = nc.dram_tensor("scA", [P, TOTC], f32, kind="ExternalInput")
    wgA_d = nc.dram_tensor("wgA", [P, TOTC], f32, kind="ExternalInput")
    scB_d = nc.dram_tensor("scB", [P, TOTC], f32, kind="ExternalInput")
    wgB_d = nc.dram_tensor("wgB", [P, TOTC], f32, kind="ExternalInput")
    outT_d = nc.dram_tensor("outT", [P, CW], bf16, kind="ExternalOutput")

    B_d = [nc.dram_tensor(f"B{s}", [BROWS, P], bf16) for s in range(NSUB)]

    with tile.TileContext(nc) as tc:
        with (
            tc.tile_pool(name="const", bufs=1) as cpool,
            tc.tile_pool(name="stagA", bufs=3) as poolA,
            tc.tile_pool(name="stagB", bufs=4) as poolB,
            tc.tile_pool(name="spool", bufs=64) as spool,
            tc.tile_pool(name="mpool", bufs=2) as mpool,
            tc.tile_pool(name="opool", bufs=2) as opool,
            tc.tile_pool(name="ipool", bufs=4) as ipool,
            tc.tile_pool(name="psA", bufs=1, space="PSUM") as psA,
            tc.tile_pool(name="psO", bufs=1, space="PSUM") as psO,
        ):
            wcat = cpool.tile([P, NUM_RELS * P], bf16)
            wroot = cpool.tile([P, P], bf16)
            biast = cpool.tile([P, 1], f32)
            iota_i = cpool.tile([P, 2 * P], i32)
            iota_f = cpool.tile([P, 2 * P], bf16)
            zrow = cpool.tile([P, P], bf16)
            scAt = cpool.tile([P, TOTC], f32)
            wgAt = cpool.tile([P, TOTC], f32)
            scBt = cpool.tile([P, TOTC], f32)
            wgBt = cpool.tile([P, TOTC], f32)
            gBt = cpool.tile([P, TOTC * P // 16], i16)
            xTt = cpool.tile([P, CW], bf16)

            nc.sync.dma_start(out=scAt[:], in_=scA_d[:])
            nc.sync.dma_start(out=wgAt[:], in_=wgA_d[:])
            nc.sync.dma_start(out=scBt[:], in_=scB_d[:])
            nc.sync.dma_start(out=wgBt[:], in_=wgB_d[:])
            nc.sync.dma_start(out=gBt[:], in_=gB_d[:])
            nc.sync.dma_start(out=wcat[:], in_=wcat_d[:])
            nc.sync.dma_start(out=wroot[:], in_=wroot_d[:])
            nc.sync.dma_start(out=biast[:], in_=bias_d[:])
            nc.sync.dma_start(out=xTt[:], in_=xT_d[:])
            nc.gpsimd.iota(iota_i[:], pattern=[[1, 2 * P]], base=0,
                           channel_multiplier=0)
            nc.vector.tensor_copy(out=iota_f[:], in_=iota_i[:])
            nc.vector.memset(zrow[:], 0.0)

            # ---- Phase A: src-window gathers -> B_s (contiguous p-major) ----
            for s in range(NSUB):
                nc.sync.dma_start(
                    out=B_d[s][NQ * CAPA:NQ * CAPA + P, :], in_=zrow[:])
                for q in range(NQ):
                    gA = ipool.tile([P, CAPA // 16], i16, tag="gA")
                    nc.sync.dma_start(out=gA[:], in_=gA_d[s * NQ + q])
                    stag = poolA.tile([P, AC, P], bf16, tag="stagA")
                    nc.gpsimd.dma_gather(
                        out_ap=stag[:],
                        in_ap=xg_d[QW * q:QW * (q + 1), :],
                        idxs_ap=gA[:],
                        num_idxs=CAPA, num_idxs_reg=CAPA, elem_size=P,
                        single_packet=False)
                    nc.sync.dma_start(
                        out=B_d[s][CAPA * q:CAPA * (q + 1), :].rearrange(
                            "(p a) d -> p a d", p=P),
                        in_=stag[:])

            # ---- Phase B: per-sweep pair gathers + segment + transform ----
            for s in range(NSWEEPS):
                sub = s // SWEEPS_PER_SUB
                swtok = NCOLS_SWEEP * P   # 2048 pair-slots
                stag = poolB.tile([P, NCOLS_SWEEP, 2 * P], bf16, tag="stagB")
                gBv = gBt[:, s * NCOLS_SWEEP * P // 16:
                          (s + 1) * NCOLS_SWEEP * P // 16]
                # overlapping pair view: idx r reads rows r, r+1 of B_d[sub]
                in_pair = bass.AP(
                    B_d[sub][:, :].tensor, 0,
                    [[P, NQ * CAPA + 1], [1, 2 * P]])
                nc.gpsimd.dma_gather(
                    out_ap=stag[:], in_ap=in_pair, idxs_ap=gBv,
                    num_idxs=swtok, num_idxs_reg=swtok, elem_size=2 * P,
                    elem_step=P,
                    single_packet=False)

                aggA = psA.tile([P, SPLIT_SLOTS], f32, tag="aggA")
                aggB = psA.tile([P, RESTB_SLOTS], f32, tag="aggB")
                ScAs = []
                ScBs = []
                for ci, (lo_t, n_t) in enumerate(SWEEP_COLS):
                    span = n_t * TILE_SLOTS
                    col = s * NCOLS_SWEEP + ci
                    ScA = spool.tile([P, 2 * P], bf16, tag="ScA")
                    ScB = spool.tile([P, 2 * P], bf16, tag="ScB")
                    ScAs.append(ScA)
                    ScBs.append(ScB)
                    nc.vector.tensor_scalar(
                        out=ScA[:, :span], in0=iota_f[:, 0:span],
                        scalar1=scAt[:, col:col + 1],
                        scalar2=wgAt[:, col:col + 1],
                        op0=mybir.AluOpType.is_equal,
                        op1=mybir.AluOpType.mult)
                    engB = nc.vector if n_t == 2 else nc.gpsimd
                    engB.tensor_scalar(
                        out=ScB[:, :span], in0=iota_f[:, 0:span],
                        scalar1=scBt[:, col:col + 1],
                        scalar2=wgBt[:, col:col + 1],
                        op0=mybir.AluOpType.is_equal,
                        op1=mybir.AluOpType.mult)
                for ci, (lo_t, n_t) in enumerate(SWEEP_COLS):
                    span = n_t * TILE_SLOTS
                    lo = lo_t * TILE_SLOTS
                    if lo_t < SPLIT_TILES:
                        aggv = aggA[:, lo:lo + span]
                    else:
                        aggv = aggB[:, lo - SPLIT_SLOTS:lo - SPLIT_SLOTS + span]
                    nc.tensor.matmul(
                        out=aggv, lhsT=stag[:, ci, 0:P],
                        rhs=ScAs[ci][:, :span],
                        start=True, stop=False)
                    nc.tensor.matmul(
                        out=aggv, lhsT=stag[:, ci, P:2 * P],
                        rhs=ScBs[ci][:, :span],
                        start=False, stop=True)

                meanA = mpool.tile([P, SPLIT_SLOTS], bf16, tag="meanA")
                meanB = mpool.tile([P, RESTB_SLOTS], bf16, tag="meanB")
                nc.scalar.activation(
                    out=meanA[:], in_=aggA[:],
                    func=mybir.ActivationFunctionType.Identity)
                nc.scalar.activation(
                    out=meanB[:], in_=aggB[:],
                    func=mybir.ActivationFunctionType.Identity)

                dst0 = s * SWEEP_DST
                outp = psO.tile([P, SWEEP_DST], f32)
                meanA_r = meanA[:].rearrange(
                    "p (dst rel) -> p dst rel", rel=NUM_RELS)
                meanB_r = meanB[:].rearrange(
                    "p (dst rel) -> p dst rel", rel=NUM_RELS)
                for r in range(NUM_RELS):
                    nc.tensor.matmul(
                        out=outp[:, :SPLIT_DST],
                        lhsT=wcat[:, r * P:(r + 1) * P],
                        rhs=meanA_r[:, :, r],
                        start=(r == 0), stop=False)
                nc.tensor.matmul(out=outp[:, :SPLIT_DST], lhsT=wroot[:],
                                 rhs=xTt[:, dst0:dst0 + SPLIT_DST],
                                 start=False, stop=True)
                for r in range(NUM_RELS):
                    nc.tensor.matmul(
                        out=outp[:, SPLIT_DST:],
                        lhsT=wcat[:, r * P:(r + 1) * P],
                        rhs=meanB_r[:, :, r],
                        start=(r == 0), stop=False)
                nc.tensor.matmul(out=outp[:, SPLIT_DST:], lhsT=wroot[:],
                                 rhs=xTt[:, dst0 + SPLIT_DST:dst0 + SWEEP_DST],
                                 start=False, stop=True)
                oT = opool.tile([P, SWEEP_DST], bf16, tag="oT")
                if s % 2 == 0:
                    nc.vector.tensor_scalar_add(
                        out=oT[:], in0=outp[:], scalar1=biast[:, 0:1])
                else:
                    nc.scalar.activation(
                        out=oT[:], in_=outp[:],
                        func=mybir.ActivationFunctionType.Identity,
                        bias=biast[:, 0:1])
                nc.sync.dma_start(out=outT_d[:, dst0:dst0 + SWEEP_DST], in_=oT[:])
    nc.compile()
    return nc


def _balance(cnt_dst):
    """Two-class LPT: top-degree dst into heavy bins (cap 128), rest into
    light bins (cap 64), 16 dst each. Returns bin_of, pos_of, loads; bins
    [0, NHBINS) heavy, rest light."""
    NHBINS = NCORES * NHEAVY
    NLBINS = NCORES * NLIGHT
    order = np.argsort(-cnt_dst, kind="stable")
    bin_of = np.empty(N_NODES, np.int64)
    pos_of = np.empty(N_NODES, np.int64)
    counts = np.zeros(NBINS, np.int32)
    loads = np.zeros(NBINS, np.int64)
    nheavy_dst = NHBINS * TILE_DST
    for part, cap in ((order[:nheavy_dst], P), (order[nheavy_dst:], 64)):
        b0 = 0 if cap == P else NHBINS
        nb = NHBINS if cap == P else NLBINS
        heap = [(0, 0, b0 + b) for b in range(nb)]
        for d in part:
            deg = int(cnt_dst[d])
            load, c, b = heapq.heappop(heap)
            assert loads[b] + deg <= cap, "two-class packing infeasible"
            bin_of[d] = b
            pos_of[d] = counts[b]
            counts[b] += 1
            loads[b] += deg
            if counts[b] < TILE_DST:
                heapq.heappush(heap, (loads[b], counts[b], b))
    return bin_of, pos_of, loads


def _prepare(x, W, W_root, bias, edge_index, edge_type):
    src = np.asarray(edge_index[0], dtype=np.int64)
    dst = np.asarray(edge_index[1], dtype=np.int64)
    rel = np.asarray(edge_type, dtype=np.int64)

    cnt_slot = np.bincount(dst * NUM_RELS + rel, minlength=N_NODES * NUM_RELS)
    w_edge = (1.0 / np.maximum(cnt_slot[dst * NUM_RELS + rel], 1)).astype(np.float32)
    cnt_dst = np.bincount(dst, minlength=N_NODES).astype(np.int64)

    bin_of, pos_of, bin_load = _balance(cnt_dst)
    NHBINS = NCORES * NHEAVY
    tile_of_bin = np.empty(NBINS, np.int64)
    core_of_bin = np.empty(NBINS, np.int64)
    hrank = np.argsort(-bin_load[:NHBINS], kind="stable")
    hslot = np.arange(NHBINS) // NCORES          # 0..NHEAVY-1
    tile_of_bin[hrank] = (hslot // HEAVY_T) * SWEEP_TILES + hslot % HEAVY_T
    core_of_bin[hrank] = np.arange(NHBINS) % NCORES
    lrank = NHBINS + np.argsort(-bin_load[NHBINS:], kind="stable")
    lslot = np.arange(NCORES * NLIGHT) // NCORES
    tile_of_bin[lrank] = ((lslot // LIGHT_T) * SWEEP_TILES + HEAVY_T
                          + lslot % LIGHT_T)
    core_of_bin[lrank] = np.arange(NCORES * NLIGHT) % NCORES

    core_of_dst = core_of_bin[bin_of]
    tile_of_dst0 = tile_of_bin[bin_of]       # heavy-first per-sweep numbering
    j_of_dst = pos_of

    e_core = core_of_dst[dst]
    e_tile0 = tile_of_dst0[dst]
    e_local = j_of_dst[dst] * NUM_RELS + rel     # slot within tile [0,128)
    q = src // QW
    e_sub = e_tile0 // TPS

    # phase A bucket caps (core, sub, q) on UNIQUE src rows
    keyA = (e_core * NSUB + e_sub) * NQ + q
    upairs = np.unique(keyA * (N_NODES + 1) + src)
    bincA = np.bincount(upairs // (N_NODES + 1), minlength=NCORES * NSUB * NQ)
    CAPA = int(-(-bincA.max() // P) * P)
    CAPA = max(CAPA, P)
    AC = CAPA // P

    xg = np.zeros((NQ * QW, P), np.float32)
    xg[:N_NODES] = np.asarray(x, np.float32)
    xg = xg.astype(BF16)
    wcat = np.ascontiguousarray(
        np.asarray(W, np.float32).transpose(1, 0, 2).reshape(P, NUM_RELS * P)
    ).astype(BF16)
    wroot = np.ascontiguousarray(np.asarray(W_root, np.float32)).astype(BF16)
    biascol = np.asarray(bias, np.float32).reshape(P, 1)

    xnp = np.asarray(x, np.float32)
    in_maps = []
    dst_tables = []
    stats = {"tokens": 0, "paired": 0}
    for c in range(NCORES):
        sel = np.nonzero(e_core == c)[0]
        csrc = src[sel]
        ctile0 = e_tile0[sel]
        clocal = e_local[sel]
        cw = w_edge[sel]
        cq = q[sel]
        csub = ctile0 // TPS

        ordT = np.lexsort((cq, csrc, ctile0))
        tT = ctile0[ordT]
        qT = cq[ordT]
        sT = csub[ordT]
        rT = csrc[ordT]
        lT = clocal[ordT]
        wT = cw[ordT]
        n = len(ordT)
        # tile group boundaries
        bnd = np.nonzero(np.diff(tT, prepend=-1))[0].tolist() + [n]

        # ---- matching: per (tile, q) pair distinct unpaired rows ----------
        partner = {}                       # (sub, q, row) -> partner row
        pair_lists = [[] for _ in range(NSUB * NQ)]
        for gi in range(len(bnd) - 1):
            a, b = bnd[gi], bnd[gi + 1]
            sb = int(sT[a])
            for qq in range(NQ):
                cand = []
                seenr = set()
                for k in range(a, b):
                    if qT[k] != qq:
                        continue
                    r = int(rT[k])
                    if r in seenr:
                        continue
                    seenr.add(r)
                    if (sb, qq, r) not in partner:
                        cand.append(r)
                for k2 in range(0, len(cand) - 1, 2):
                    r1, r2 = cand[k2], cand[k2 + 1]
                    partner[(sb, qq, r1)] = r2
                    partner[(sb, qq, r2)] = r1
                    pair_lists[sb * NQ + qq].append((r1, r2))

        # ---- B_d row order per bucket: pairs adjacent, then the rest ------
        brow_of = {}                       # (sub, q, row) -> brow within sub
        gA = np.zeros((NSUB * NQ, P, CAPA // 16), np.int16)
        for sb in range(NSUB):
            for qq in range(NQ):
                bidx = sb * NQ + qq
                m = (csub == sb) & (cq == qq)
                uniq = np.unique(csrc[m])
                inpair = set()
                ordered = []
                for (r1, r2) in pair_lists[bidx]:
                    ordered.append(r1)
                    ordered.append(r2)
                    inpair.add(r1)
                    inpair.add(r2)
                for r in uniq.tolist():
                    if r not in inpair:
                        ordered.append(r)
                nrow = len(ordered)
                assert nrow == len(uniq) and nrow <= CAPA, (nrow, len(uniq), CAPA)
                for jpos, r in enumerate(ordered):
                    brow_of[(sb, qq, r)] = CAPA * qq + jpos
                arr = np.zeros(CAPA, np.int64)
                arr[:nrow] = np.asarray(ordered, np.int64) - QW * qq
                ii = np.arange(CAPA)
                jj = (ii % P) * AC + ii // P
                gA[bidx] = _wrap16(arr[jj].astype(np.int16))

        # ---- slots per tile: pair-slots + singles -------------------------
        slots_by_tile = {}   # tile0 -> list of (browA, (lA,wA), (lB,wB)|None)
        for gi in range(len(bnd) - 1):
            a, b = bnd[gi], bnd[gi + 1]
            t0 = int(tT[a])
            sb = int(sT[a])
            toks = {}
            for k in range(a, b):
                toks.setdefault((int(qT[k]), int(rT[k])), []).append(k)
            slots = []
            done = set()
            for (qq, r), ks in toks.items():
                if (qq, r) in done:
                    continue
                pr = partner.get((sb, qq, r))
                mate = toks.get((qq, pr)) if pr is not None else None
                if mate is not None and (qq, pr) not in done:
                    b1 = brow_of[(sb, qq, r)]
                    b2 = brow_of[(sb, qq, pr)]
                    if b2 == b1 + 1:
                        lowks, hiks, blo = ks, mate, b1
                    elif b1 == b2 + 1:
                        lowks, hiks, blo = mate, ks, b2
                    else:
                        lowks = None
                    if lowks is not None:
                        npair = min(len(lowks), len(hiks))
                        for t in range(npair):
                            ka, kb = lowks[t], hiks[t]
                            slots.append((blo, (int(lT[ka]), float(wT[ka])),
                                          (int(lT[kb]), float(wT[kb]))))
                        for k2 in lowks[npair:]:
                            slots.append((blo, (int(lT[k2]), float(wT[k2])),
                                          None))
                        for k2 in hiks[npair:]:
                            slots.append((blo + 1 - 1 + 1,
                                          (int(lT[k2]), float(wT[k2])), None))
                        done.add((qq, r))
                        done.add((qq, pr))
                        stats["paired"] += 2 * npair
                        continue
                for k2 in ks:
                    slots.append((brow_of[(sb, qq, r)],
                                  (int(lT[k2]), float(wT[k2])), None))
                done.add((qq, r))
            slots_by_tile[t0] = slots
        stats["tokens"] += n

        # ---- per sweep: assign tiles to fixed columns, renumber -----------
        newtile_of_old = np.empty(NTILES, np.int64)
        gB_cols = np.full((TOTC, P), NQ * CAPA, np.int64)
        scA_cols = np.zeros((TOTC, P), np.float32)
        wgA_cols = np.zeros((TOTC, P), np.float32)
        scB_cols = np.zeros((TOTC, P), np.float32)
        wgB_cols = np.zeros((TOTC, P), np.float32)
        for s in range(NSWEEPS):
            base = s * SWEEP_TILES
            hv = list(range(base, base + HEAVY_T))
            lt = list(range(base + HEAVY_T, base + SWEEP_TILES))
            nsl = {t: len(slots_by_tile.get(t, [])) for t in hv + lt}
            hv.sort(key=lambda t: -nsl[t])
            lt.sort(key=lambda t: nsl[t])
            # 12 (H,L) pairs: biggest H with smallest L; 4 smallest H single
            pairs = [(hv[i], lt[i]) for i in range(LIGHT_T)]
            singles = hv[LIGHT_T:]
            for i, (th, tl) in enumerate(pairs):
                assert nsl[th] + nsl[tl] <= P, (s, nsl[th], nsl[tl])
            # column order: sideA = pairs 0..5, sideB = pairs 6..11 + singles
            coltiles = [list(pr) for pr in pairs] + [[t] for t in singles]
            newt = base
            for ci, ts in enumerate(coltiles):
                col = s * NCOLS_SWEEP + ci
                pslot = 0
                for ti, t in enumerate(ts):
                    newtile_of_old[t] = newt
                    newt += 1
                    for (browA, (la, wa), bpart) in slots_by_tile.get(t, []):
                        gB_cols[col, pslot] = browA
                        scA_cols[col, pslot] = ti * TILE_SLOTS + la
                        wgA_cols[col, pslot] = wa
                        if bpart is not None:
                            lb, wb = bpart
                            scB_cols[col, pslot] = ti * TILE_SLOTS + lb
                            wgB_cols[col, pslot] = wb
                        pslot += 1
                assert pslot <= P
            assert newt == base + SWEEP_TILES

        gB16 = _wrap16(gB_cols.reshape(-1).astype(np.int16))
        scA_arr = np.ascontiguousarray(scA_cols.T)
        wgA_arr = np.ascontiguousarray(wgA_cols.T)
        scB_arr = np.ascontiguousarray(scB_cols.T)
        wgB_arr = np.ascontiguousarray(wgB_cols.T)

        # ---- dst table with renumbered tiles ------------------------------
        mask = core_of_dst == c
        dst_ids = np.nonzero(mask)[0]
        cols_d = (newtile_of_old[tile_of_dst0[dst_ids]] * TILE_DST
                  + j_of_dst[dst_ids])
        dst_table = np.full(CW, -1, np.int64)
        dst_table[cols_d] = dst_ids
        valid = dst_table >= 0
        xT = np.zeros((P, CW), np.float32)
        xT[:, valid] = xnp[dst_table[valid]].T
        xT = xT.astype(BF16)

        in_maps.append({
            "xg": xg,
            "xT": xT,
            "wcat": wcat,
            "wroot": wroot,
            "bias": biascol,
            "gA": gA,
            "gB": gB16,
            "scA": scA_arr,
            "wgA": wgA_arr,
            "scB": scB_arr,
            "wgB": wgB_arr,
        })
        dst_tables.append(dst_table)
    return in_maps, dst_tables, CAPA, stats


LAST_EXEC_NS = None


def kernel(x, W, W_root, bias, edge_index, edge_type):
    global _compiled, LAST_EXEC_NS
    import os
    from concourse.bass_utils import run_bass_kernel_spmd

    in_maps, dst_tables, CAPA, stats = _prepare(
        x, W, W_root, bias, edge_index, edge_type)
    key = CAPA
    if _compiled is None or _compiled[0] != key:
        nc = _build_program(CAPA)
        _compiled = (key, nc)
    nc = _compiled[1]

    trace = bool(int(os.environ.get("BASS_PROFILE", "0")))
    r = run_bass_kernel_spmd(nc, in_maps, list(range(NCORES)), trace=trace)
    if trace and getattr(r, "exec_time_ns", None) is not None:
        LAST_EXEC_NS = r.exec_time_ns
    res = r.results
    out = np.empty((N_NODES, DIM), np.float32)
    for c in range(NCORES):
        outT = np.asarray(res[c]["outT"]).astype(np.float32)
        dt = dst_tables[c]
        valid = dt >= 0
        out[dt[valid]] = outT[:, valid].T
    return out
